# revision 27
# baseline (speedup 1.0000x reference)
"""DeepFuseMamba2 fusion block on 8 trn2 NeuronCores.

Sharding: data-parallel over batch B=8 -> one image per core.
Per-core pipeline, processed in 16-row H-strips (matmuls bf16, fp32 PSUM):
  HWC load (SWDGE cast-dma fp32->bf16) -> xbar DMA-transpose to CHW ->
  conv1x1 (PE, bias on ACT evict) -> depthwise 3x3 as 9 diag-matmul taps
  accumulated in PSUM with border-clipped APs -> V back to HWC via xbar ->
  per-row cross attention (logits PE; exp + row-sum fused on ACT accum_out;
  softmax scale folded into reciprocal; normalize DVE; attn^T via xbar) ->
  output projection with down/lp2/rp2/beta/gamma algebraically folded into
  4 PSUM-accumulated matmuls -> xbar back to HWC -> cast-dma store fp32.
No softmax max-subtraction: |logits| < 1 for this model scale (verified).
"""

import os
import numpy as np
import ml_dtypes

import concourse.bass as bass
from concourse import bacc
import concourse.mybir as mybir
import concourse.tile as tile
from concourse import bass_utils

BF16 = mybir.dt.bfloat16
F32 = mybir.dt.float32

B, C, H, W = 8, 96, 256, 256
HW = H * W
R = 16                     # rows per strip
S = H // R                 # strips per image
SCALE = float(C) ** -0.5

# tap order: center first so the start=True matmul covers the full region
TAPS = [(0, 0), (-1, -1), (-1, 0), (-1, 1), (0, -1), (0, 1), (1, -1), (1, 0), (1, 1)]


def build_nc(n_strips=S):
    nc = bacc.Bacc()

    I1 = nc.dram_tensor("I1", [HW, C], F32, kind="ExternalInput")
    I2 = nc.dram_tensor("I2", [HW, C], F32, kind="ExternalInput")
    w1T = nc.dram_tensor("w1T", [4, C, C], BF16, kind="ExternalInput")
    wdg = nc.dram_tensor("wdg", [4 * 9, C, C], BF16, kind="ExternalInput")
    wfT = nc.dram_tensor("wfT", [4, C, C], BF16, kind="ExternalInput")
    b1 = nc.dram_tensor("b1", [C, 4], F32, kind="ExternalInput")
    bd = nc.dram_tensor("bd", [C, 4], F32, kind="ExternalInput")
    bf = nc.dram_tensor("bf", [C, 1], F32, kind="ExternalInput")
    bfld = nc.dram_tensor("bfld", [C, 36], F32, kind="ExternalInput")
    OUT = nc.dram_tensor("OUT", [HW, C], F32, kind="ExternalOutput")

    ins = [I1, I2]

    with tile.TileContext(nc) as tc:
        with (
            tc.tile_pool(name="const", bufs=1) as const,
            tc.tile_pool(name="xh", bufs=1) as xh_pool,
            tc.tile_pool(name="xt", bufs=2) as xt_pool,
            tc.tile_pool(name="qq", bufs=2) as qq_pool,
            tc.tile_pool(name="qv", bufs=1) as qv_pool,
            tc.tile_pool(name="vh", bufs=2) as vh_pool,
            tc.tile_pool(name="fp", bufs=2) as f_pool,
            tc.tile_pool(name="fu", bufs=1) as fu_pool,
            tc.tile_pool(name="oh", bufs=2) as oh_pool,
            tc.tile_pool(name="sm", bufs=4) as sm_pool,
            tc.tile_pool(name="ea", bufs=4) as ea_pool,
            tc.tile_pool(name="pw", bufs=2, space="PSUM") as pw_pool,
            tc.tile_pool(name="pl", bufs=2, space="PSUM") as pl_pool,
            tc.tile_pool(name="pf", bufs=2, space="PSUM") as pf_pool,
        ):
            # ---- constants ----
            w1_sb = const.tile([C, 4, C], BF16)
            nc.gpsimd.dma_start(out=w1_sb, in_=w1T.rearrange("p a b -> a p b"))
            wdg_sb = const.tile([C, 36, C], BF16)
            nc.gpsimd.dma_start(out=wdg_sb, in_=wdg.rearrange("p a b -> a p b"))
            wf_sb = const.tile([C, 4, C], BF16)
            nc.gpsimd.dma_start(out=wf_sb, in_=wfT.rearrange("p a b -> a p b"))
            b1_sb = const.tile([C, 4], F32)
            nc.gpsimd.dma_start(out=b1_sb, in_=b1[:, :])
            bd_sb = const.tile([C, 4], F32)
            nc.gpsimd.dma_start(out=bd_sb, in_=bd[:, :])
            bf_sb = const.tile([C, 1], F32)
            nc.gpsimd.dma_start(out=bf_sb, in_=bf[:, :])
            bfld_sb = const.tile([C, 36], F32)
            nc.gpsimd.dma_start(out=bfld_sb, in_=bfld[:, :])

            for s in range(n_strips):
                h0 = s * R
                # buffer row i (0..R+1) = image row h0 - 1 + i
                i_lo = 1 if s == 0 else 0
                i_hi = R + 1 if s == S - 1 else R + 2
                px_lo = (h0 - 1 + i_lo) * W
                npix = (i_hi - i_lo) * W
                nblk = npix // 128

                # ---- load HWC strips (fp32 -> bf16 cast dma) ----
                xh = []
                for t, inp in enumerate(ins):
                    xt_h = xh_pool.tile([128, (R + 2) * 2, 128], BF16, tag=f"xh{t}")
                    src = inp[px_lo:px_lo + npix, :].rearrange(
                        "(k p) c -> p k c", p=128)
                    nc.gpsimd.dma_start(out=xt_h[:, i_lo * 2:i_lo * 2 + nblk, 0:C],
                                        in_=src)
                    xh.append(xt_h)

                # ---- transpose HWC -> CHW ----
                xt = []
                for t in range(2):
                    x_t = xt_pool.tile([128, R + 2, W], BF16, tag=f"xt{t}")
                    dst = x_t[:, i_lo:i_hi, :].rearrange(
                        "c r (q p) -> c (r q) p", p=128)
                    nc.sync.dma_start(
                        out=dst, in_=xh[t][:, i_lo * 2:i_lo * 2 + nblk, :],
                        transpose=True)
                    xt.append(x_t)

                # ---- conv1x1 + dwconv3 for the 4 projections ----
                qv = []
                for p in range(4):
                    xsrc = xt[0] if p < 2 else xt[1]
                    q_t = (qq_pool if p in (0, 2) else qv_pool).tile(
                        [C, R, W], BF16, tag=f"qv{p}")
                    for r0 in range(0, R, 2):
                        ps = pw_pool.tile([C, 2, W], F32, tag="pw")
                        for k, (dh, dw) in enumerate(TAPS):
                            # valid out rows r in chunk: 0 <= h0+r+dh < H
                            r_a = max(r0, -(h0 + dh))
                            r_b = min(r0 + 2, H - h0 - dh)
                            if r_b <= r_a:
                                continue
                            ic0, oc0, ncol = (0, 1, W - 1) if dw == -1 else \
                                             ((1, 0, W - 1) if dw == 1 else (0, 0, W))
                            nc.tensor.matmul(
                                ps[:, r_a - r0:r_b - r0, oc0:oc0 + ncol],
                                wdg_sb[:, p * 9 + k, :],
                                xsrc[:C, r_a + 1 + dh:r_b + 1 + dh,
                                     ic0:ic0 + ncol],
                                start=(k == 0), stop=(k == len(TAPS) - 1))
                        # evict with per-region bias (vert 0/1/2, horz 0/1/2)
                        row_groups = []
                        for r in (r0, r0 + 1):
                            vi = 0 if h0 + r == 0 else (2 if h0 + r == H - 1
                                                        else 1)
                            if row_groups and row_groups[-1][2] == vi:
                                row_groups[-1][1] = r + 1
                            else:
                                row_groups.append([r, r + 1, vi])
                        for ra, rb, vi in row_groups:
                            base = p * 9 + vi * 3
                            nc.scalar.activation(
                                out=q_t[:, ra:rb, 1:W - 1],
                                in_=ps[:, ra - r0:rb - r0, 1:W - 1],
                                func=mybir.ActivationFunctionType.Identity,
                                bias=bfld_sb[:, base + 1:base + 2], scale=1.0)
                            nc.vector.tensor_scalar_add(
                                out=q_t[:, ra:rb, 0:1],
                                in0=ps[:, ra - r0:rb - r0, 0:1],
                                scalar1=bfld_sb[:, base:base + 1])
                            nc.vector.tensor_scalar_add(
                                out=q_t[:, ra:rb, W - 1:W],
                                in0=ps[:, ra - r0:rb - r0, W - 1:W],
                                scalar1=bfld_sb[:, base + 2:base + 3])
                    qv.append(q_t)

                # ---- V tensors CHW -> HWC ----
                vh = []
                for t, p in ((0, 1), (1, 3)):
                    v_t = vh_pool.tile([128, 2 * R, C], BF16, tag=f"vh{t}")
                    nc.sync.dma_start(out=v_t, in_=qv[p], transpose=True)
                    vh.append(v_t)

                # ---- per-row cross attention ----
                f1_t = f_pool.tile([C, R, W], BF16, tag="f1")
                f2_t = f_pool.tile([C, R, W], BF16, tag="f2")
                for r in range(R):
                    pl = pl_pool.tile([128, 512], F32, tag="pl")
                    for m in range(2):
                        nc.tensor.matmul(pl[:, m * 256:m * 256 + 256],
                                         qv[0][:, r, m * 128:m * 128 + 128],
                                         qv[2][:, r, :])
                    e_t = ea_pool.tile([128, 512], BF16, tag="e")
                    rs = sm_pool.tile([128, 4], F32, tag="rs")
                    for m in range(2):
                        nc.scalar.activation(
                            out=e_t[:, m * 256:m * 256 + 256],
                            in_=pl[:, m * 256:m * 256 + 256],
                            func=mybir.ActivationFunctionType.Exp,
                            accum_out=rs[:, m:m + 1])
                    rc = sm_pool.tile([128, 4], F32, tag="rc")
                    nc.vector.reciprocal(rc[:, 0:2], rs[:, 0:2])
                    nc.vector.tensor_scalar_mul(rc[:, 2:4], in0=rc[:, 0:2],
                                                scalar1=SCALE)
                    a_t = ea_pool.tile([128, 512], BF16, tag="a")
                    for m in range(2):
                        nc.vector.tensor_scalar_mul(
                            a_t[:, m * 256:m * 256 + 256],
                            in0=e_t[:, m * 256:m * 256 + 256],
                            scalar1=rc[:, 2 + m:3 + m])
                    at_t = ea_pool.tile([128, 2, 256], BF16, tag="at")
                    for m in range(2):
                        nc.sync.dma_start(
                            out=at_t[:, :, m * 128:m * 128 + 128],
                            in_=a_t[:, m * 256:m * 256 + 256], transpose=True)
                    pf1 = pf_pool.tile([C, 512], F32, tag="pf1")
                    for vb in range(2):
                        nc.tensor.matmul(pf1[:, 0:256], vh[1][:, 2 * r + vb, :],
                                         at_t[:, vb, :],
                                         start=(vb == 0), stop=(vb == 1))
                    nc.vector.tensor_copy(out=f1_t[:, r, :], in_=pf1[:, 0:256])
                    pf2 = pf_pool.tile([C, 512], F32, tag="pf2")
                    for m in range(2):
                        nc.tensor.matmul(pf2[:, 0:256], vh[0][:, 2 * r + m, :],
                                         a_t[:, m * 256:m * 256 + 256],
                                         start=(m == 0), stop=(m == 1))
                    nc.vector.tensor_copy(out=f2_t[:, r, :], in_=pf2[:, 0:256])

                # ---- fused output projection ----
                fu_t = fu_pool.tile([C, R, W], BF16, tag="fu")
                for r0 in range(0, R, 2):
                    ps = pw_pool.tile([C, 2, W], F32, tag="pw")
                    ops = [(wf_sb[:, 0, :], xt[0][:C, r0 + 1:r0 + 3, :]),
                           (wf_sb[:, 1, :], f1_t[:, r0:r0 + 2, :]),
                           (wf_sb[:, 2, :], xt[1][:C, r0 + 1:r0 + 3, :]),
                           (wf_sb[:, 3, :], f2_t[:, r0:r0 + 2, :])]
                    for k, (lhs, rhs) in enumerate(ops):
                        nc.tensor.matmul(ps, lhs, rhs, start=(k == 0),
                                         stop=(k == 3))
                    nc.scalar.activation(
                        out=fu_t[:, r0:r0 + 2, :], in_=ps,
                        func=mybir.ActivationFunctionType.Identity,
                        bias=bf_sb[:, 0:1], scale=1.0)

                # ---- CHW -> HWC and store ----
                o_t = oh_pool.tile([128, 2 * R, C], BF16, tag="oh")
                nc.sync.dma_start(out=o_t, in_=fu_t, transpose=True)
                dst = OUT[h0 * W:(h0 + R) * W, :].rearrange(
                    "(k p) c -> p k c", p=128)
                nc.gpsimd.dma_start(out=dst, in_=o_t)

    nc.finalize()
    return nc


def prep_weights(se1_w, se1_b, se1_dw, se1_db, se2_w, se2_b, se2_dw, se2_db,
                 lp1_w, lp1_b, lp1_dw, lp1_db, rp1_w, rp1_b, rp1_dw, rp1_db,
                 lp2_w, lp2_b, rp2_w, rp2_b, down_w, down_b, beta, gamma):
    bf = ml_dtypes.bfloat16
    convs = [(se1_w, se1_b, se1_dw, se1_db), (lp1_w, lp1_b, lp1_dw, lp1_db),
             (se2_w, se2_b, se2_dw, se2_db), (rp1_w, rp1_b, rp1_dw, rp1_db)]
    w1T = np.stack([w.T for (w, _, _, _) in convs]).astype(bf)
    # fused conv1x1*dwconv tap matrices, lhsT layout [c_in, c_out]:
    # out[o,p] = sum_taps dwk[o,tap] * (W1 @ x_shift)[o,p]
    wdg = np.zeros((36, C, C), np.float32)
    for p, (w1, _, dwk, _) in enumerate(convs):
        k9 = dwk.reshape(C, 3, 3)
        for k, (dh, dw) in enumerate(TAPS):
            wdg[p * 9 + k] = (w1 * k9[:, dh + 1, dw + 1][:, None]).T
    wdg = wdg.astype(bf)
    b1 = np.stack([b for (_, b, _, _) in convs], axis=1).astype(np.float32)
    bd = np.stack([b for (_, _, _, b) in convs], axis=1).astype(np.float32)
    # border bias fields: Bf[p, vert, horz][o] = bd + b1*sum(valid dwk taps)
    # vert/horz: 0=edge at start (top/left), 1=interior, 2=edge at end
    bfld = np.zeros((C, 36), np.float32)
    for p, (_, b1v, dwk, bdv) in enumerate(convs):
        k9 = dwk.reshape(C, 3, 3)
        for vi, vs in enumerate((slice(1, 3), slice(0, 3), slice(0, 2))):
            for hi, hs in enumerate((slice(1, 3), slice(0, 3), slice(0, 2))):
                S = k9[:, vs, hs].sum(axis=(1, 2))
                bfld[:, p * 9 + vi * 3 + hi] = bdv + b1v * S

    beta_c = beta.reshape(C)
    gamma_c = gamma.reshape(C)
    DWl, DWr = down_w[:, :C], down_w[:, C:]
    ML = DWl @ (beta_c[:, None] * lp2_w)
    MR = DWr @ (gamma_c[:, None] * rp2_w)
    wfT = np.stack([DWl.T, ML.T, DWr.T, MR.T]).astype(bf)
    bfuse = (down_b + DWl @ (beta_c * lp2_b) + DWr @ (gamma_c * rp2_b))
    bfuse = bfuse.astype(np.float32).reshape(C, 1)
    return dict(w1T=w1T, wdg=wdg, wfT=wfT, b1=b1, bd=bd, bf=bfuse,
                bfld=bfld)


_cache = {}
last_exec_time_ns = None


def kernel(I1, I2, h, w, **kw):
    global last_exec_time_ns
    I1 = np.asarray(I1, np.float32)
    I2 = np.asarray(I2, np.float32)
    wts = prep_weights(**{k: np.asarray(v, np.float32) for k, v in kw.items()})
    if "nc" not in _cache:
        _cache["nc"] = build_nc()
    nc = _cache["nc"]
    in_maps = [dict(I1=np.ascontiguousarray(I1[b]),
                    I2=np.ascontiguousarray(I2[b]), **wts) for b in range(B)]
    trace = bool(int(os.environ.get("DFM_TRACE", "0")))
    tmpdir = os.environ.get("DFM_TRACE_DIR") or None
    res = bass_utils.run_bass_kernel_spmd(nc, in_maps, core_ids=list(range(B)),
                                          trace=trace, tmpdir=tmpdir)
    if trace:
        last_exec_time_ns = res.exec_time_ns
    out = np.stack([res.results[b]["OUT"] for b in range(B)])
    return out.astype(np.float32)
